# revision 13
# baseline (speedup 1.0000x reference)
"""Segment-mean + tiny classifier (ALLonBert post-encoder) on 8 TRN2 cores.

Data-parallel over batch: each core handles 2 of the 16 rows (8192
tokens). The host quantizes hidden to fp8e4 with per-segment error
feedback (noise shaping): within each segment the running quantization
residual is added to the next token before rounding, so the segment-sum
error telescopes to a single final residual instead of growing like
sqrt(n). Measured end-to-end rel err ~5e-3 (vs 2.5e-2 for plain fp8).

The 8.4 MiB/core fp8 stream halves the bf16 baseline's HBM traffic and
rides both HWDGE rings as 16 x 512 KiB single-chunk DMAs (even chunks
on sync, odd on scalar) so arrivals alternate and TensorE starts ~3 us
in. Tokens are laid out four-per-partition (4 KiB descriptor lines).
Per 512-token chunk, one-hot tiles built on DVE (is_equal vs iota)
feed matmuls accumulating segment sums into PSUM [128 segs, 2 x 512].

VARIANT env knob (bisect aid):
  dr   (default) fp8 + DoubleRow perf mode, 256-token contraction
  fp8  fp8, plain matmul, 128-token contraction
  bf16 bf16 hidden + one-hot, plain matmul

The last chunks issue all bank-0 matmuls first so ps0 closes early;
the classifier then runs as four chained tensor_tensor_reduce on DVE
(fused multiply+reduce, initial-value chaining across banks) partially
overlapped with the final bank-1 matmuls, followed by a split
scale/bias on DVE+Pool and a single store.
"""

import os
import sys

if "/opt/trn_rl_repo" not in sys.path:
    sys.path.insert(0, "/opt/trn_rl_repo")

import numpy as np

B, S, H = 16, 4096, 1024
NSEG = 64
SEP_ID = 102
NCORES = 8
RPC = B // NCORES          # batch rows per core
T = RPC * S                # tokens per core
NC4 = T // 512             # 512-token chunks (4 tokens per partition)
MSEG = RPC * NSEG          # output segments per core (= 128)

VARIANT = os.environ.get("KERNEL_VARIANT", "dr")
CLS = os.environ.get("KERNEL_CLS", "amr")

_CACHE: dict = {}


def _build():
    if "nc" in _CACHE:
        return _CACHE["nc"]
    from concourse import bacc, tile, mybir
    import concourse.bass as bass

    f32 = mybir.dt.float32
    bf16 = mybir.dt.bfloat16
    f8 = mybir.dt.float8e4
    hdt = bf16 if VARIANT == "bf16" else f8
    Al = mybir.AluOpType
    DR = mybir.MatmulPerfMode.DoubleRow

    nc = bacc.Bacc(None, target_bir_lowering=False, debug=False)
    hidden = nc.declare_dram_parameter("hidden", [T, H], hdt, isOutput=False)
    seg = nc.declare_dram_parameter("seg", [128, 4 * NC4], f32, isOutput=False)
    w2 = nc.declare_dram_parameter("w2", [128, 2 * H], bf16, isOutput=False)
    invb = nc.declare_dram_parameter("invb", [128, 4], f32, isOutput=False)
    out = nc.declare_dram_parameter("out", [128, 128], f32, isOutput=True)

    # pair view: token t = (pp*2 + u)*512 + p*4 + j -> two 4 KiB lines
    # per partition per 1 MiB DMA
    hp = hidden[:].rearrange("(pp u p j) h -> pp p u j h", pp=NC4 // 2, u=2, p=128, j=4)

    with tile.TileContext(nc) as tc:
        with (
            tc.tile_pool(name="const", bufs=1) as cpool,
            tc.tile_pool(name="hid", bufs=NC4 // 2) as hpool,
            tc.tile_pool(name="mt", bufs=16) as mpool,
            tc.tile_pool(name="psum", bufs=1, space=bass.MemorySpace.PSUM) as ppool,
        ):
            ps0 = ppool.tile([128, 512], f32)
            ps1 = ppool.tile([128, 512], f32)

            # seg gates the one-hot builds -- first bytes on the sync ring
            # (the scalar ring's first transfer starts ~1.8 us later)
            seg_t = cpool.tile([128, 4 * NC4], f32)
            nc.sync.dma_start(seg_t[:], seg[:])
            invb_t = cpool.tile([128, 4], f32)
            nc.sync.dma_start(invb_t[:], invb[:])

            # hidden stream: 1 MiB chunk-pairs alternating rings; w2 rides
            # scalar BEFORE the final pair so only hidden trails the stream
            pair_ts = []
            w_t = cpool.tile([128, 2 * H], bf16)
            for pp in range(NC4 // 2):
                hid_t = hpool.tile([128, 2, 4, 1024], hdt, tag="hid")
                eng = nc.sync if pp % 2 == 0 else nc.scalar
                if pp == NC4 // 2 - 1:
                    nc.scalar.dma_start(w_t[:], w2[:])
                eng.dma_start(hid_t[:], hp[pp])
                pair_ts.append(hid_t)

            def hslice(c, jlo, jhi, hf):
                return pair_ts[c // 2][:, c % 2, jlo:jhi, 512 * hf : 512 * hf + 512]

            iota_t = cpool.tile([128, 128], f32)
            nc.gpsimd.iota(
                iota_t[:],
                pattern=[[1, 128]],
                base=0,
                channel_multiplier=0,
                allow_small_or_imprecise_dtypes=True,
            )
            logit = cpool.tile([128, 128], f32)
            nc.gpsimd.memset(logit[:], 0.0)

            NLAST = 2  # trailing chunks run bank-0-first so ps0 closes early

            if VARIANT == "dr":
                def mk_mts(c):
                    # two [128, 2, 128] one-hot tiles: token (c, p, 2*pr+q)
                    # -> col block q of tile pr
                    mts = []
                    for pr in range(2):
                        mt_t = mpool.tile([128, 2, 128], f8)
                        for q in range(2):
                            col = 4 * c + 2 * pr + q
                            nc.vector.tensor_scalar(
                                mt_t[:, q, :], iota_t[:],
                                seg_t[:, col : col + 1], None, op0=Al.is_equal,
                            )
                        mts.append(mt_t)
                    return mts

                def chunk_mm(c, mts, hf, ps, start, stop):
                    for pr in range(2):
                        nc.tensor.matmul(
                            ps[:],
                            mts[pr][:],
                            hslice(c, 2 * pr, 2 * pr + 2, hf),
                            start=start and pr == 0,
                            stop=stop and pr == 1,
                            perf_mode=DR,
                        )
            else:
                def mk_mts(c):
                    # four [128, 128] one-hot tiles, one per token slot j
                    mts = []
                    for j in range(4):
                        mt_t = mpool.tile([128, 128], hdt)
                        col = 4 * c + j
                        nc.vector.tensor_scalar(
                            mt_t[:], iota_t[:],
                            seg_t[:, col : col + 1], None, op0=Al.is_equal,
                        )
                        mts.append(mt_t)
                    return mts

                def chunk_mm(c, mts, hf, ps, start, stop):
                    for j in range(4):
                        nc.tensor.matmul(
                            ps[:],
                            mts[j][:],
                            pair_ts[c // 2][:, c % 2, j,
                                            512 * hf : 512 * hf + 512],
                            start=start and j == 0,
                            stop=stop and j == 3,
                        )

            for c in range(NC4 - NLAST):
                mts = mk_mts(c)
                for hf in range(2):
                    chunk_mm(c, mts, hf, ps0 if hf == 0 else ps1,
                             start=(c == 0), stop=False)
            last_mts = {c: mk_mts(c) for c in range(NC4 - NLAST, NC4)}
            for hf in range(2):
                ps = ps0 if hf == 0 else ps1
                for c in range(NC4 - NLAST, NC4):
                    chunk_mm(c, last_mts[c], hf, ps,
                             start=False, stop=(c == NC4 - 1))

            # classifier: logits[s, c] = invcnt[s] * sum_h sums[s,h] W[c,h] + b_c
            # as four chained fused multiply+reduce on DVE; the ps0 pair
            # overlaps the final bank-1 matmuls
            if CLS == "amr":
                # fused (ps * invcnt) . w per class+bank on DVE; the ps0
                # pair overlaps the final bank-1 matmuls. scale=invcnt AP
                # folds the mean division in; combine halves + bias in one
                # tensor_scalar per class.
                scr = cpool.tile([128, 512], f32)
                scr2 = cpool.tile([128, 512], f32)
                acc = cpool.tile([128, 4], f32)
                nc.vector.affine_mul_reduce(
                    scr[:], acc[:, 0:1], ps0[:], w_t[:, 0:512],
                    invb_t[:, 0:1], 0.0,
                )
                nc.vector.affine_mul_reduce(
                    scr2[:], acc[:, 1:2], ps0[:], w_t[:, 1024:1536],
                    invb_t[:, 0:1], 0.0,
                )
                nc.vector.affine_mul_reduce(
                    scr[:], acc[:, 2:3], ps1[:], w_t[:, 512:1024],
                    invb_t[:, 0:1], 0.0,
                )
                nc.vector.affine_mul_reduce(
                    scr2[:], acc[:, 3:4], ps1[:], w_t[:, 1536:2048],
                    invb_t[:, 0:1], 0.0,
                )
                # logit_c = (a_c0 + a_c1) + bias_c
                nc.vector.tensor_scalar(
                    logit[:, 0:1], acc[:, 0:1], acc[:, 2:3], invb_t[:, 1:2],
                    op0=Al.add, op1=Al.add,
                )
                nc.gpsimd.tensor_scalar(
                    logit[:, 1:2], acc[:, 1:2], acc[:, 3:4], invb_t[:, 2:3],
                    op0=Al.add, op1=Al.add,
                )
                nc.sync.dma_start(out[:], logit[:])
            elif CLS == "ttr":
                scr = cpool.tile([128, 512], f32)
                scr2 = cpool.tile([128, 512], f32)
                acc = cpool.tile([128, 4], f32)
                nc.vector.tensor_tensor_reduce(
                    scr[:], ps0[:], w_t[:, 0:512], 1.0, 0.0,
                    op0=Al.mult, op1=Al.add, accum_out=acc[:, 0:1],
                )
                nc.vector.tensor_tensor_reduce(
                    scr2[:], ps0[:], w_t[:, 1024:1536], 1.0, 0.0,
                    op0=Al.mult, op1=Al.add, accum_out=acc[:, 1:2],
                )
                nc.vector.tensor_tensor_reduce(
                    scr[:], ps1[:], w_t[:, 512:1024], 1.0, acc[:, 0:1],
                    op0=Al.mult, op1=Al.add, accum_out=acc[:, 2:3],
                )
                nc.vector.tensor_tensor_reduce(
                    scr2[:], ps1[:], w_t[:, 1536:2048], 1.0, acc[:, 1:2],
                    op0=Al.mult, op1=Al.add, accum_out=acc[:, 3:4],
                )
                a0, a1 = acc[:, 2:3], acc[:, 3:4]
            else:
                # baseline-style: products into scr [cls, bank] then reduce
                scr = cpool.tile([128, 2048], f32)
                acc = cpool.tile([128, 2], f32)
                nc.vector.tensor_tensor(
                    scr[:, 0:512], ps0[:], w_t[:, 0:512], op=Al.mult
                )
                nc.vector.tensor_tensor(
                    scr[:, 1024:1536], ps0[:], w_t[:, 1024:1536], op=Al.mult
                )
                nc.vector.tensor_tensor(
                    scr[:, 512:1024], ps1[:], w_t[:, 512:1024], op=Al.mult
                )
                nc.vector.tensor_tensor(
                    scr[:, 1536:2048], ps1[:], w_t[:, 1536:2048], op=Al.mult
                )
                nc.vector.tensor_reduce(
                    acc[:],
                    scr[:].rearrange("p (c x) -> p c x", c=2),
                    axis=mybir.AxisListType.X,
                    op=Al.add,
                )
                a0, a1 = acc[:, 0:1], acc[:, 1:2]
            if CLS != "amr":
                nc.vector.tensor_scalar(
                    logit[:, 0:1], a0, invb_t[:, 0:1], invb_t[:, 1:2],
                    op0=Al.mult, op1=Al.add,
                )
                nc.gpsimd.tensor_scalar(
                    logit[:, 1:2], a1, invb_t[:, 0:1], invb_t[:, 2:3],
                    op0=Al.mult, op1=Al.add,
                )
                nc.sync.dma_start(out[:], logit[:])

    nc.compile()
    _CACHE["nc"] = nc
    return nc


def _host_prep(hidden_states, classifier_w, classifier_b, input_ids):
    import ml_dtypes

    f8 = ml_dtypes.float8_e4m3
    ids = np.asarray(input_ids)
    sep = ids == SEP_ID
    seg = np.cumsum(sep, axis=1) - sep.astype(np.int64)          # [B, S]
    pos = np.arange(S)
    num_seps = sep.sum(axis=1, keepdims=True)
    valid = (~sep) & (pos[None, :] >= 1) & (seg < num_seps)      # [B, S]

    counts = np.zeros((B, NSEG), np.float32)
    for b in range(B):
        cb = np.bincount(seg[b][valid[b]], minlength=NSEG)[:NSEG]
        counts[b] = cb
    cnt = np.maximum(counts, 1.0)                                # [B, NSEG]

    flat = np.where(valid, seg, -1).astype(np.int64)             # [B, S]

    hs = np.asarray(hidden_states, dtype=np.float32)
    if VARIANT == "bf16":
        q8 = (hs * valid[..., None]).astype(ml_dtypes.bfloat16)
    else:
        # error-feedback fp8 quantization: carry the rounding residual
        # along valid tokens so each segment sum's error telescopes
        q8 = np.empty((B, S, H), dtype=f8)
        r = np.zeros((B, H), np.float32)
        zero8 = np.zeros((H,), dtype=f8)
        for s in range(S):
            v = valid[:, s]
            t = hs[:, s] + np.where(v[:, None], r, 0.0)
            q = t.astype(f8)
            q8[:, s] = np.where(v[:, None], q, zero8)
            r = np.where(v[:, None], t - q.astype(np.float32), r)

    W = np.asarray(classifier_w, dtype=np.float32)
    bvec = np.asarray(classifier_b, dtype=np.float32)
    w2 = np.ascontiguousarray(
        np.broadcast_to(W.reshape(1, 2 * H), (128, 2 * H))
    ).astype(ml_dtypes.bfloat16)

    in_maps = []
    for i in range(NCORES):
        rows = slice(RPC * i, RPC * (i + 1))
        fl = flat[rows].copy()                                   # [RPC, S]
        for rr in range(RPC):
            m = fl[rr] >= 0
            fl[rr][m] += rr * NSEG
        flt = fl.reshape(T)                                      # [T]
        # token order: t = c*512 + 4p + j -> seg_param[p, 4*c + j]
        sp = flt.reshape(NC4, 128, 4)                            # [c, p, j]
        seg_param = np.ascontiguousarray(
            sp.transpose(1, 0, 2).reshape(128, 4 * NC4).astype(np.float32)
        )
        cvec = cnt[rows].reshape(MSEG)                           # [128]
        invb = np.ascontiguousarray(
            np.stack(
                [1.0 / cvec, np.full(MSEG, bvec[0]), np.full(MSEG, bvec[1]),
                 np.zeros(MSEG, np.float32)],
                axis=1,
            ).astype(np.float32)
        )                                                        # [128, 4]
        in_maps.append(
            {
                "hidden": np.ascontiguousarray(q8[rows].reshape(T, H)),
                "seg": seg_param,
                "w2": w2,
                "invb": invb,
            }
        )
    return in_maps


def kernel(hidden_states, classifier_w, classifier_b, input_ids, n_segs):
    from concourse.bass_utils import run_bass_kernel_spmd

    nc = _build()
    in_maps = _host_prep(hidden_states, classifier_w, classifier_b, input_ids)
    res = run_bass_kernel_spmd(nc, in_maps, core_ids=list(range(NCORES)))
    outs = [
        res.results[i]["out"][:, 0:2].reshape(RPC, NSEG, 2)
        for i in range(NCORES)
    ]
    return np.concatenate(outs, axis=0).astype(np.float32)
